# revision 6
# baseline (speedup 1.0000x reference)
"""Attention-pooling kernel for Trainium2 (raw Bass), SPMD over 8 NeuronCores.

Computation (per sample b):
    score[t] = tanh(sum_d X[b,t,d] * w[d] + bias[t])
    out[b,d] = sum_t softmax_t(score)[t] * X[b,t,d]

Sharding: data-parallel over batch (32 samples -> 4 per core); w/bias
replicated. X uploaded fp16 (t-rows permuted t = p*16 + c so per-sample DMA
slices are contiguous 16KB-per-partition runs; softmax/pooling are
t-permutation invariant, bias is loaded with the same permutation).

v3 design (HW-measured on this container's trn2 cores):
  - The 16-chunk score multiply+reduce per sample is split across THREE
    engines (HW-probed costs: DVE stt ~1.0-1.13us/chunk fused; DVE TT 2x
    ~0.5us + ACT Identity-accum ~1.1-1.23us; Pool TT mult ~1.74us/chunk):
      g0: stt{0-4} on DVE, Pool-TT{5,6}, DVE-TT{7}
      g1: stt{8-11} on DVE, Pool-TT{12,13,14}, DVE-TT{15}
    ACT reduces all Pool/DVE TT products (Identity with bias/D folded in as
    the per-element ACT bias). Balances DVE ~10.3 / ACT ~9.8 / Pool ~8.7 us
    per sample against the HW DMA floor of ~10.9us/sample (~390 GB/s/core,
    measured: the 16MiB fp16 X load is the hard roofline; neither SWDGE vs
    HWDGE issue nor deeper buffering changes it).
  - Input DMA issue moved from Pool (busy computing now) to the SP engine,
    which does NOTHING else (an output DMA on the SP queue would gate input
    issue on the downstream pipeline — measured as full serialization in the
    timeline sim). Output DMAs are issued from the ACT queue right after the
    copies that produce them, gated on the copies' completion semaphore.
  - 4-deep xt buffering (the issue->transfer->score->pool loop spans ~36us;
    3 buffers would cap the pipeline at 47us/pass, below the DMA rate).
  - score/esc/sumexp/recip triple-buffered so softmax(v) need not wait for
    pooling(v-2), only pooling(v-3).
  - Pooling interleaved per group on PE as in v2 (16 matmuls after each
    group's exp); 1/sum(exp) folded into the ACT PSUM->SBUF output copies
    as the per-partition scale.
  - An explicit self-semaphore wait remains between DVE stt and the next
    DVE reader of its accum_out (accum lands after retire; race confirmed
    on HW by the v2 session).
Steady state targets the DMA roofline ~44us per pass of 4 samples (v2: 57us).
"""

import numpy as np

import concourse.bass as bass
import concourse.mybir as mybir
from concourse.bass_utils import run_bass_kernel_spmd

B, T, D = 32, 2048, 1024
N_CORES = 8
BPC = B // N_CORES  # samples per core
P = 128
NCHUNK = T // P  # 16
NGROUP = 2
CPG = NCHUNK // NGROUP  # 8

f32 = mybir.dt.float32
fp16 = mybir.dt.float16
Tanh = mybir.ActivationFunctionType.Tanh
Exp = mybir.ActivationFunctionType.Exp
Copy = mybir.ActivationFunctionType.Copy
Identity = mybir.ActivationFunctionType.Identity
Alu = mybir.AluOpType

# per-group chunk assignment: S on DVE (fused stt), Q on Pool (TT mult),
# C on DVE (TT mult); Q and C products reduced on ACT.
S0, Q0, C0 = [0, 1, 2, 3, 4], [5, 6], [7]
S1, Q1, C1 = [8, 9, 10, 11], [12, 13, 14], [15]
NQ = len(Q0) + len(Q1)  # Pool chunks / sample
NC = len(C0) + len(C1)  # DVE-TT chunks / sample
PSLOT = 6  # Pool product ring depth
DSLOT = 3  # DVE product ring depth
XBUF = 4  # xt buffers
SBUF3 = 3  # score/esc/sumexp/recip buffers


def _build_nc(reps: int = 1) -> bass.Bass:
    nc = bass.Bass("TRN2", target_bir_lowering=False, debug=False)
    x = nc.dram_tensor("x", [BPC, T, D], fp16, kind="ExternalInput").ap()
    w = nc.dram_tensor("w", [D, 1], fp16, kind="ExternalInput")
    bias = nc.dram_tensor("bias", [T, 1], f32, kind="ExternalInput")
    bias_s = nc.dram_tensor("bias_s", [T, 1], f32, kind="ExternalInput")  # bias/D
    out = nc.dram_tensor("out", [BPC, D], f32, kind="ExternalOutput").ap()

    NS = BPC * reps

    from contextlib import ExitStack

    with ExitStack() as es:
        ec = es.enter_context
        xt = [ec(nc.sbuf_tensor(f"xt{i}", [P, NCHUNK, D], fp16)) for i in range(XBUF)]
        wt = ec(nc.sbuf_tensor("wt", [P, D], fp16))
        bias_t = ec(nc.sbuf_tensor("bias_t", [P, NCHUNK], f32))
        bias_st = ec(nc.sbuf_tensor("bias_st", [P, NCHUNK], f32))
        ones_col = ec(nc.sbuf_tensor("ones_col", [P, 1], f32))
        prodp = ec(nc.sbuf_tensor("prodp", [P, PSLOT, D], fp16))
        prodd = ec(nc.sbuf_tensor("prodd", [P, DSLOT, D], fp16))
        scrap = ec(nc.sbuf_tensor("scrap", [P, NCHUNK], fp16))
        score = [
            ec(nc.sbuf_tensor(f"score{i}", [P, NCHUNK], f32)) for i in range(SBUF3)
        ]
        esc = [ec(nc.sbuf_tensor(f"esc{i}", [P, NCHUNK], fp16)) for i in range(SBUF3)]
        sumexp = [
            ec(nc.sbuf_tensor(f"sumexp{i}", [P, NGROUP], f32)) for i in range(SBUF3)
        ]
        recip = [ec(nc.sbuf_tensor(f"recip{i}", [1, 1], f32)) for i in range(SBUF3)]
        orow = [ec(nc.sbuf_tensor(f"orow{i}", [1, D], f32)) for i in range(2)]
        pa = [ec(nc.psum_tensor(f"pool_a{i}", [1, 512], f32)) for i in range(2)]
        pb = [ec(nc.psum_tensor(f"pool_b{i}", [1, 512], f32)) for i in range(2)]
        tot = [ec(nc.psum_tensor(f"tot{i}", [1, 1], f32)) for i in range(2)]

        cset = ec(nc.semaphore("cset"))
        ones_sem = ec(nc.semaphore("ones_sem"))
        dma_in_s = [
            [ec(nc.semaphore(f"dma_in{b_}{g}")) for g in range(NGROUP)]
            for b_ in range(XBUF)
        ]
        poolp = ec(nc.semaphore("poolp"))  # Pool TT products done
        dved = ec(nc.semaphore("dved"))  # DVE TT products done
        sttb = ec(nc.semaphore("sttb"))  # DVE stt accum landed
        qb = ec(nc.semaphore("qb"))  # bias added per group
        actq = ec(nc.semaphore("actq"))  # ACT reduces of Pool products
        actc = ec(nc.semaphore("actc"))  # ACT reduces of DVE products
        act_g = ec(nc.semaphore("act_g"))  # exp done (per group: 2/sample)
        pe_tot = ec(nc.semaphore("pe_tot"))  # tot matmul done (1/sample)
        recip_sem = ec(nc.semaphore("recip_sem"))  # recip ready (1/sample)
        act_out = ec(nc.semaphore("act_out"))  # output copies done (1/sample)
        pe_pool = ec(nc.semaphore("pe_pool"))  # pooling done (1/sample)
        dma_out_s = [ec(nc.semaphore(f"dma_out{i}")) for i in range(2)]
        block = ec(nc.Block())

        def k_of(v):  # how many times xt slot v%XBUF has been (re)loaded
            return v // XBUF + 1

        @block.sync
        def _(sync):
            sync.dma_start(
                wt[:], bass.AP(tensor=w, offset=0, ap=[[0, P], [1, D]])
            ).then_inc(cset, 16)
            bap = [[NCHUNK, P], [1, NCHUNK]]  # bias_t[p, c] = bias[p*16 + c]
            sync.dma_start(
                bias_t[:], bass.AP(tensor=bias, offset=0, ap=bap)
            ).then_inc(cset, 16)
            sync.dma_start(
                bias_st[:], bass.AP(tensor=bias_s, offset=0, ap=bap)
            ).then_inc(cset, 16)
            for v in range(NS):
                s4 = v % XBUF
                if v >= XBUF:
                    sync.wait_ge(pe_pool, v - XBUF + 1)  # xt slot free
                xs = x[v % BPC].rearrange("(p c) d -> p c d", c=NCHUNK)
                for g in range(NGROUP):
                    sync.dma_start(
                        out=xt[s4][:, g * CPG : (g + 1) * CPG, :],
                        in_=xs[:, g * CPG : (g + 1) * CPG, :],
                    ).then_inc(dma_in_s[s4][g], 16)
            sync.wait_ge(dma_out_s[0], 16 * ((NS + 1) // 2))
            if NS > 1:
                sync.wait_ge(dma_out_s[1], 16 * (NS // 2))

        @block.gpsimd
        def _(gpsimd):
            gpsimd.memset(ones_col[:], 1.0).then_inc(ones_sem, 1)
            gpsimd.wait_ge(cset, 48)
            kq = 0
            for v in range(NS):
                s4 = v % XBUF
                for g, qs in ((0, Q0), (1, Q1)):
                    gpsimd.wait_ge(dma_in_s[s4][g], 16 * k_of(v))
                    for c in qs:
                        if kq >= PSLOT:
                            gpsimd.wait_ge(actq, kq - PSLOT + 1)
                        gpsimd.tensor_tensor(
                            out=prodp[:, kq % PSLOT, :],
                            in0=xt[s4][:, c, :],
                            in1=wt[:],
                            op=Alu.mult,
                        ).then_inc(poolp, 1)
                        kq += 1

        @block.vector
        def _(vector):
            vector.wait_ge(cset, 48)
            k_stt = 0
            kc = 0

            def recip_of(u):
                u3 = u % SBUF3
                vector.wait_ge(pe_tot, u + 1)
                if u >= SBUF3:
                    vector.wait_ge(act_out, u - SBUF3 + 1)  # recip[u3] free
                nc.vector.reciprocal(out=recip[u3][:], in_=tot[u % 2][:]).then_inc(
                    recip_sem, 1
                )

            for v in range(NS):
                s4, p3 = v % XBUF, v % SBUF3
                if v >= SBUF3:
                    # score[p3] free: exp g1 of sample v-3 done
                    vector.wait_ge(act_g, NGROUP * (v - SBUF3) + 2)
                for g, ss, cs in ((0, S0, C0), (1, S1, C1)):
                    vector.wait_ge(dma_in_s[s4][g], 16 * k_of(v))
                    for c in ss:
                        nc.vector.scalar_tensor_tensor(
                            out=scrap[:, c : c + 1].broadcast_to((P, D)),
                            in0=xt[s4][:, c, :],
                            scalar=0.0,
                            in1=wt[:],
                            op0=Alu.bypass,
                            op1=Alu.mult,
                            accum_out=score[p3][:, c : c + 1],
                        ).then_inc(sttb, 1)
                        k_stt += 1
                    for c in cs:
                        if kc >= DSLOT:
                            vector.wait_ge(actc, kc - DSLOT + 1)
                        nc.vector.tensor_tensor(
                            out=prodd[:, kc % DSLOT, :],
                            in0=xt[s4][:, c, :],
                            in1=wt[:],
                            op=Alu.mult,
                        ).then_inc(dved, 1)
                        kc += 1
                    # accum_out lands after retire; must see our own writes
                    vector.wait_ge(sttb, k_stt)
                    qs = slice(ss[0], ss[-1] + 1)
                    nc.vector.tensor_tensor(
                        out=score[p3][:, qs],
                        in0=score[p3][:, qs],
                        in1=bias_t[:, qs],
                        op=Alu.add,
                    ).then_inc(qb, 1)
                    if g == 0 and v >= 1:
                        recip_of(v - 1)
            recip_of(NS - 1)

        @block.scalar
        def _(scalar):
            scalar.wait_ge(cset, 48)
            kq = 0
            kc = 0

            def copies_and_out(u):
                u2, u3, s = u % 2, u % SBUF3, u % BPC
                scalar.wait_ge(pe_pool, u + 1)
                scalar.wait_ge(recip_sem, u + 1)
                if u >= 2:
                    scalar.wait_ge(dma_out_s[u2], 16 * (u // 2))  # orow[u2] free
                nc.scalar.activation(
                    out=orow[u2][:, 0:512], in_=pa[u2][:], func=Copy,
                    scale=recip[u3][:],
                )
                nc.scalar.activation(
                    out=orow[u2][:, 512:1024], in_=pb[u2][:], func=Copy,
                    scale=recip[u3][:],
                ).then_inc(act_out, 1)
                # HWDGE issue is SEQ-side: wait for the copies' completion sem
                # before the DMA reads orow.
                scalar.wait_ge(act_out, u + 1)
                scalar.dma_start(out=out[s : s + 1, :], in_=orow[u2][:]).then_inc(
                    dma_out_s[u2], 16
                )

            for v in range(NS):
                p3 = v % SBUF3
                if v >= SBUF3:
                    scalar.wait_ge(pe_pool, v - 2)  # esc/sumexp[p3] free
                for g, qs_, cs in ((0, Q0, C0), (1, Q1, C1)):
                    for c in qs_:
                        scalar.wait_ge(poolp, kq + 1)
                        nc.scalar.activation(
                            out=prodp[:, kq % PSLOT, :],
                            in_=prodp[:, kq % PSLOT, :],
                            func=Identity,
                            bias=bias_st[:, c : c + 1],
                            accum_out=score[p3][:, c : c + 1],
                        ).then_inc(actq, 1)
                        kq += 1
                    for c in cs:
                        scalar.wait_ge(dved, kc + 1)
                        nc.scalar.activation(
                            out=prodd[:, kc % DSLOT, :],
                            in_=prodd[:, kc % DSLOT, :],
                            func=Identity,
                            bias=bias_st[:, c : c + 1],
                            accum_out=score[p3][:, c : c + 1],
                        ).then_inc(actc, 1)
                        kc += 1
                    gs = slice(g * CPG, (g + 1) * CPG)
                    scalar.wait_ge(qb, NGROUP * v + g + 1)
                    nc.scalar.activation(
                        out=score[p3][:, gs], in_=score[p3][:, gs], func=Tanh
                    )
                    nc.scalar.activation(
                        out=esc[p3][:, gs],
                        in_=score[p3][:, gs],
                        func=Exp,
                        accum_out=sumexp[p3][:, g : g + 1],
                    ).then_inc(act_g, 1)
                if v >= 1:
                    copies_and_out(v - 1)
            copies_and_out(NS - 1)

        @block.tensor
        def _(tensor):
            tensor.wait_ge(ones_sem, 1)
            for v in range(NS):
                s4, p3, p2 = v % XBUF, v % SBUF3, v % 2
                for g in range(NGROUP):
                    tensor.wait_ge(act_g, NGROUP * v + g + 1)
                    if g == 0:
                        if v >= 2:
                            tensor.wait_ge(recip_sem, v - 1)  # tot[p2] free
                            tensor.wait_ge(act_out, v - 1)  # pa/pb[p2] free
                    mm_t = nc.tensor.matmul(
                        tot[p2][:],
                        sumexp[p3][:, g : g + 1],
                        ones_col[:],
                        start=(g == 0),
                        stop=(g == NGROUP - 1),
                    )
                    if g == NGROUP - 1:
                        mm_t.then_inc(pe_tot, 1)
                    for c in range(g * CPG, (g + 1) * CPG):
                        st, sp = c == 0, c == NCHUNK - 1
                        nc.tensor.matmul(
                            pa[p2][:], esc[p3][:, c : c + 1], xt[s4][:, c, 0:512],
                            start=st, stop=sp,
                        )
                        mm = nc.tensor.matmul(
                            pb[p2][:], esc[p3][:, c : c + 1], xt[s4][:, c, 512:1024],
                            start=st, stop=sp,
                        )
                mm.then_inc(pe_pool, 1)

    return nc


_NC_CACHE: dict = {}


def _build(reps: int = 1, **kw) -> bass.Bass:
    key = (reps, tuple(sorted(kw.items())))
    if key not in _NC_CACHE:
        _NC_CACHE[key] = _build_nc(reps, **kw)
    return _NC_CACHE[key]


def _in_maps(x, w, b):
    x16 = np.asarray(x, dtype=np.float16)
    w16 = np.ascontiguousarray(np.asarray(w, dtype=np.float16))
    b = np.ascontiguousarray(np.asarray(b, dtype=np.float32))
    return [
        {
            "x": x16[c * BPC : (c + 1) * BPC],
            "w": w16,
            "bias": b,
            "bias_s": b / D,
        }
        for c in range(N_CORES)
    ]


def kernel(**inputs):
    x = np.asarray(inputs["inputs"], dtype=np.float32)
    w = np.asarray(inputs["att_weight"], dtype=np.float32)
    b = np.asarray(inputs["att_bias"], dtype=np.float32)
    nc = _build()
    res = run_bass_kernel_spmd(nc, _in_maps(x, w, b), list(range(N_CORES)))
    return np.concatenate([r["out"] for r in res.results], axis=0)


# revision 10
# speedup vs baseline: 1.5615x; 1.5615x over previous
"""Attention-pooling kernel for Trainium2 (raw Bass), SPMD over 8 NeuronCores.

Computation (per sample b):
    score[t] = tanh(sum_d X[b,t,d] * w[d] + bias[t])
    out[b,d] = sum_t softmax_t(score)[t] * X[b,t,d]

Sharding: data-parallel over batch (32 samples -> 4 per core); w/bias
replicated. X uploaded fp16 (t-rows permuted t = p*16 + c so per-sample DMA
slices are contiguous 16KB-per-partition runs; softmax/pooling are
t-permutation invariant, bias is loaded with the same permutation).

v3 design (HW-measured on this container's trn2 cores):
  - The 16-chunk score multiply+reduce per sample is split across THREE
    engines (HW-probed costs: DVE stt ~1.0-1.13us/chunk fused; DVE TT 2x
    ~0.5us + ACT Identity-accum ~1.1-1.23us; Pool TT mult ~1.74us/chunk):
      g0: stt{0-4} on DVE, Pool-TT{5,6}, DVE-TT{7}
      g1: stt{8-11} on DVE, Pool-TT{12,13,14}, DVE-TT{15}
    ACT reduces all Pool/DVE TT products (Identity with bias/D folded in as
    the per-element ACT bias). Balances DVE ~10.3 / ACT ~9.8 / Pool ~8.7 us
    per sample against the HW DMA floor of ~10.9us/sample (~390 GB/s/core,
    measured: the 16MiB fp16 X load is the hard roofline; neither SWDGE vs
    HWDGE issue nor deeper buffering changes it).
  - Input DMA issue moved from Pool (busy computing now) to the SP engine,
    which does NOTHING else (an output DMA on the SP queue would gate input
    issue on the downstream pipeline — measured as full serialization in the
    timeline sim). Output DMAs are issued from the ACT queue right after the
    copies that produce them, gated on the copies' completion semaphore.
  - 4-deep xt buffering (the issue->transfer->score->pool loop spans ~36us;
    3 buffers would cap the pipeline at 47us/pass, below the DMA rate).
  - score/esc/sumexp/recip triple-buffered so softmax(v) need not wait for
    pooling(v-2), only pooling(v-3).
  - Pooling interleaved per group on PE as in v2 (16 matmuls after each
    group's exp); 1/sum(exp) folded into the ACT PSUM->SBUF output copies
    as the per-partition scale.
  - An explicit self-semaphore wait remains between DVE stt and the next
    DVE reader of its accum_out (accum lands after retire; race confirmed
    on HW by the v2 session).
Steady state targets the DMA roofline ~44us per pass of 4 samples (v2: 57us).
"""

import numpy as np

import concourse.bass as bass
import concourse.mybir as mybir
from concourse.bass_utils import run_bass_kernel_spmd

B, T, D = 32, 2048, 1024
N_CORES = 8
BPC = B // N_CORES  # samples per core
P = 128
NCHUNK = T // P  # 16
NGROUP = 2
CPG = NCHUNK // NGROUP  # 8

f32 = mybir.dt.float32
fp16 = mybir.dt.float16
Tanh = mybir.ActivationFunctionType.Tanh
Exp = mybir.ActivationFunctionType.Exp
Copy = mybir.ActivationFunctionType.Copy
Identity = mybir.ActivationFunctionType.Identity
Alu = mybir.AluOpType

# per-group chunk assignment: S on DVE (fused stt), Q on Pool (TT mult),
# C on DVE (TT mult); Q and C products reduced on ACT.
S0, Q0, C0 = [0, 1, 2, 3, 4], [5, 6], [7]
S1, Q1, C1 = [8, 9, 10, 11], [12, 13, 14], [15]
NQ = len(Q0) + len(Q1)  # Pool chunks / sample
NC = len(C0) + len(C1)  # DVE-TT chunks / sample
PSLOT = 6  # Pool product ring depth
DSLOT = 3  # DVE product ring depth
DSLOT_NP = 10  # DVE ring depth when pool_q disabled
XBUF = 4  # xt buffers
SBUF3 = 3  # score/esc/sumexp/recip buffers


def _build_nc(reps: int = 1, pool_q: bool = False, act_dma: bool = True, in_dma: bool = True) -> bass.Bass:
    nc = bass.Bass("TRN2", target_bir_lowering=False, debug=False)
    x = nc.dram_tensor("x", [BPC, T, D], fp16, kind="ExternalInput").ap()
    w = nc.dram_tensor("w", [D, 1], fp16, kind="ExternalInput")
    bias = nc.dram_tensor("bias", [T, 1], f32, kind="ExternalInput")
    bias_s = nc.dram_tensor("bias_s", [T, 1], f32, kind="ExternalInput")  # bias/D
    out = nc.dram_tensor("out", [BPC, D], f32, kind="ExternalOutput").ap()

    NS = BPC * reps
    if pool_q:
        s0, q0, c0, s1, q1, c1 = S0, Q0, C0, S1, Q1, C1
        dslot = DSLOT
    else:
        s0, q0, c0 = [0, 1, 2, 3], [], [4, 5, 6, 7]
        s1, q1, c1 = [8, 9, 10], [], [11, 12, 13, 14, 15]
        dslot = DSLOT_NP

    from contextlib import ExitStack

    with ExitStack() as es:
        ec = es.enter_context
        xt = [ec(nc.sbuf_tensor(f"xt{i}", [P, NCHUNK, D], fp16)) for i in range(XBUF)]
        wt = ec(nc.sbuf_tensor("wt", [P, D], fp16))
        bias_t = ec(nc.sbuf_tensor("bias_t", [P, NCHUNK], f32))
        bias_st = ec(nc.sbuf_tensor("bias_st", [P, NCHUNK], f32))
        ones_col = ec(nc.sbuf_tensor("ones_col", [P, 1], f32))
        prodp = ec(nc.sbuf_tensor("prodp", [P, PSLOT, D], fp16))
        prodd = ec(nc.sbuf_tensor("prodd", [P, dslot, D], fp16))
        scrap = ec(nc.sbuf_tensor("scrap", [P, NCHUNK], fp16))
        score = [
            ec(nc.sbuf_tensor(f"score{i}", [P, NCHUNK], f32)) for i in range(SBUF3)
        ]
        esc = [ec(nc.sbuf_tensor(f"esc{i}", [P, NCHUNK], fp16)) for i in range(SBUF3)]
        sumexp = [
            ec(nc.sbuf_tensor(f"sumexp{i}", [P, NGROUP], f32)) for i in range(SBUF3)
        ]
        recip = [ec(nc.sbuf_tensor(f"recip{i}", [1, 1], f32)) for i in range(SBUF3)]
        orow = [ec(nc.sbuf_tensor(f"orow{i}", [1, BPC * D], f32)) for i in range(2)]
        pa = [ec(nc.psum_tensor(f"pool_a{i}", [1, 512], f32)) for i in range(2)]
        pb = [ec(nc.psum_tensor(f"pool_b{i}", [1, 512], f32)) for i in range(2)]
        tot = [ec(nc.psum_tensor(f"tot{i}", [1, 1], f32)) for i in range(2)]

        cset = ec(nc.semaphore("cset"))
        ones_sem = ec(nc.semaphore("ones_sem"))
        dma_in_s = [
            [ec(nc.semaphore(f"dma_in{b_}{g}")) for g in range(NGROUP)]
            for b_ in range(XBUF)
        ]
        poolp = ec(nc.semaphore("poolp"))  # Pool TT products done
        dved = ec(nc.semaphore("dved"))  # DVE TT products done
        sttb = ec(nc.semaphore("sttb"))  # DVE stt accum landed
        qb = ec(nc.semaphore("qb"))  # bias added per group
        actq = ec(nc.semaphore("actq"))  # ACT reduces of Pool products
        actc = ec(nc.semaphore("actc"))  # ACT reduces of DVE products
        act_g = ec(nc.semaphore("act_g"))  # exp done (per group: 2/sample)
        pe_tot = ec(nc.semaphore("pe_tot"))  # tot matmul done (1/sample)
        recip_sem = ec(nc.semaphore("recip_sem"))  # recip ready (1/sample)
        act_out = ec(nc.semaphore("act_out"))  # output copies done (1/sample)
        pe_pool = ec(nc.semaphore("pe_pool"))  # pooling done (1/sample)
        dma_out_s = [ec(nc.semaphore(f"dma_out{i}")) for i in range(2)]
        block = ec(nc.Block())

        def k_of(v):  # how many times xt slot v%XBUF has been (re)loaded
            return (v // XBUF + 1) if in_dma else 1

        @block.sync
        def _(sync):
            sync.dma_start(
                wt[:], bass.AP(tensor=w, offset=0, ap=[[0, P], [1, D]])
            ).then_inc(cset, 16)
            bap = [[NCHUNK, P], [1, NCHUNK]]  # bias_t[p, c] = bias[p*16 + c]
            sync.dma_start(
                bias_t[:], bass.AP(tensor=bias, offset=0, ap=bap)
            ).then_inc(cset, 16)
            sync.dma_start(
                bias_st[:], bass.AP(tensor=bias_s, offset=0, ap=bap)
            ).then_inc(cset, 16)
            for v in range(NS if in_dma else min(NS, XBUF)):
                s4 = v % XBUF
                if v >= XBUF:
                    sync.wait_ge(pe_pool, v - XBUF + 1)  # xt slot free
                xs = x[v % BPC].rearrange("(p c) d -> p c d", c=NCHUNK)
                for g in range(NGROUP):
                    sync.dma_start(
                        out=xt[s4][:, g * CPG : (g + 1) * CPG, :],
                        in_=xs[:, g * CPG : (g + 1) * CPG, :],
                    ).then_inc(dma_in_s[s4][g], 16)
                if v % BPC == 0 and v >= 2 * BPC:
                    r = v // BPC - 2  # rep-wide output, 2-rep lookback
                    sync.wait_ge(act_out, BPC * (r + 1))
                    sync.dma_start(out=out[:, :], in_=orow[r % 2][:]).then_inc(
                        dma_out_s[r % 2], 16
                    )
            for r in range(max(reps - 2, 0) if in_dma else 0, reps):
                sync.wait_ge(act_out, BPC * (r + 1))
                sync.dma_start(out=out[:, :], in_=orow[r % 2][:]).then_inc(
                    dma_out_s[r % 2], 16
                )
            sync.wait_ge(dma_out_s[0], 16 * ((reps + 1) // 2))
            if reps > 1:
                sync.wait_ge(dma_out_s[1], 16 * (reps // 2))

        @block.gpsimd
        def _(gpsimd):
            gpsimd.memset(ones_col[:], 1.0).then_inc(ones_sem, 1)
            gpsimd.wait_ge(cset, 48)
            kq = 0
            for v in range(NS):
                s4 = v % XBUF
                for g, qs in ((0, q0), (1, q1)):
                    gpsimd.wait_ge(dma_in_s[s4][g], 16 * k_of(v))
                    for c in qs:
                        if kq >= PSLOT:
                            gpsimd.wait_ge(actq, kq - PSLOT + 1)
                        gpsimd.tensor_tensor(
                            out=prodp[:, kq % PSLOT, :],
                            in0=xt[s4][:, c, :],
                            in1=wt[:],
                            op=Alu.mult,
                        ).then_inc(poolp, 1)
                        kq += 1

        @block.vector
        def _(vector):
            vector.wait_ge(cset, 48)
            k_stt = 0
            kc = 0

            def recip_of(u):
                u3 = u % SBUF3
                vector.wait_ge(pe_tot, u + 1)
                if u >= SBUF3:
                    vector.wait_ge(act_out, u - SBUF3 + 1)  # recip[u3] free
                nc.vector.reciprocal(out=recip[u3][:], in_=tot[u % 2][:]).then_inc(
                    recip_sem, 1
                )

            for v in range(NS):
                s4, p3 = v % XBUF, v % SBUF3
                if v >= SBUF3:
                    # score[p3] free: exp g1 of sample v-3 done
                    vector.wait_ge(act_g, NGROUP * (v - SBUF3) + 2)
                for g, ss, cs in ((0, s0, c0), (1, s1, c1)):
                    vector.wait_ge(dma_in_s[s4][g], 16 * k_of(v))
                    for c in ss:
                        nc.vector.scalar_tensor_tensor(
                            out=scrap[:, c : c + 1].broadcast_to((P, D)),
                            in0=xt[s4][:, c, :],
                            scalar=0.0,
                            in1=wt[:],
                            op0=Alu.bypass,
                            op1=Alu.mult,
                            accum_out=score[p3][:, c : c + 1],
                        ).then_inc(sttb, 1)
                        k_stt += 1
                    for c in cs:
                        if kc >= dslot:
                            vector.wait_ge(actc, kc - dslot + 1)
                        nc.vector.tensor_tensor(
                            out=prodd[:, kc % dslot, :],
                            in0=xt[s4][:, c, :],
                            in1=wt[:],
                            op=Alu.mult,
                        ).then_inc(dved, 1)
                        kc += 1
                    # accum_out lands after retire; must see our own writes
                    vector.wait_ge(sttb, k_stt)
                    qs = slice(ss[0], ss[-1] + 1)
                    nc.vector.tensor_tensor(
                        out=score[p3][:, qs],
                        in0=score[p3][:, qs],
                        in1=bias_t[:, qs],
                        op=Alu.add,
                    ).then_inc(qb, 1)
                    if g == 0 and v >= 1:
                        recip_of(v - 1)
            recip_of(NS - 1)

        @block.scalar
        def _(scalar):
            scalar.wait_ge(cset, 48)
            kq = 0
            kc = 0

            def copies_and_out(u):
                u2, u3, s = u % 2, u % SBUF3, u % BPC
                r, rp = u // BPC, (u // BPC) % 2
                o0 = s * D
                scalar.wait_ge(pe_pool, u + 1)
                scalar.wait_ge(recip_sem, u + 1)
                if s == 0 and r >= 2:
                    # orow[rp] free: its rep-(r-2) DMA done
                    scalar.wait_ge(dma_out_s[rp], 16 * (r // 2))
                nc.scalar.activation(
                    out=orow[rp][:, o0 : o0 + 512], in_=pa[u2][:], func=Copy,
                    scale=recip[u3][:],
                )
                nc.scalar.activation(
                    out=orow[rp][:, o0 + 512 : o0 + 1024], in_=pb[u2][:], func=Copy,
                    scale=recip[u3][:],
                ).then_inc(act_out, 1)

            for v in range(NS):
                p3 = v % SBUF3
                if v >= SBUF3:
                    scalar.wait_ge(pe_pool, v - 2)  # esc/sumexp[p3] free
                for g, qs_, cs in ((0, q0, c0), (1, q1, c1)):
                    for c in qs_:
                        scalar.wait_ge(poolp, kq + 1)
                        nc.scalar.activation(
                            out=prodp[:, kq % PSLOT, :],
                            in_=prodp[:, kq % PSLOT, :],
                            func=Identity,
                            bias=bias_st[:, c : c + 1],
                            accum_out=score[p3][:, c : c + 1],
                        ).then_inc(actq, 1)
                        kq += 1
                    for c in cs:
                        scalar.wait_ge(dved, kc + 1)
                        nc.scalar.activation(
                            out=prodd[:, kc % dslot, :],
                            in_=prodd[:, kc % dslot, :],
                            func=Identity,
                            bias=bias_st[:, c : c + 1],
                            accum_out=score[p3][:, c : c + 1],
                        ).then_inc(actc, 1)
                        kc += 1
                    gs = slice(g * CPG, (g + 1) * CPG)
                    scalar.wait_ge(qb, NGROUP * v + g + 1)
                    nc.scalar.activation(
                        out=score[p3][:, gs], in_=score[p3][:, gs], func=Tanh
                    )
                    nc.scalar.activation(
                        out=esc[p3][:, gs],
                        in_=score[p3][:, gs],
                        func=Exp,
                        accum_out=sumexp[p3][:, g : g + 1],
                    ).then_inc(act_g, 1)
                if v >= 1:
                    copies_and_out(v - 1)
            copies_and_out(NS - 1)

        @block.tensor
        def _(tensor):
            tensor.wait_ge(ones_sem, 1)
            for v in range(NS):
                s4, p3, p2 = v % XBUF, v % SBUF3, v % 2
                for g in range(NGROUP):
                    tensor.wait_ge(act_g, NGROUP * v + g + 1)
                    if g == 0:
                        if v >= 2:
                            tensor.wait_ge(recip_sem, v - 1)  # tot[p2] free
                            tensor.wait_ge(act_out, v - 1)  # pa/pb[p2] free
                    mm_t = nc.tensor.matmul(
                        tot[p2][:],
                        sumexp[p3][:, g : g + 1],
                        ones_col[:],
                        start=(g == 0),
                        stop=(g == NGROUP - 1),
                    )
                    if g == NGROUP - 1:
                        mm_t.then_inc(pe_tot, 1)
                    for c in range(g * CPG, (g + 1) * CPG):
                        st, sp = c == 0, c == NCHUNK - 1
                        nc.tensor.matmul(
                            pa[p2][:], esc[p3][:, c : c + 1], xt[s4][:, c, 0:512],
                            start=st, stop=sp,
                        )
                        mm = nc.tensor.matmul(
                            pb[p2][:], esc[p3][:, c : c + 1], xt[s4][:, c, 512:1024],
                            start=st, stop=sp,
                        )
                mm.then_inc(pe_pool, 1)

    return nc


_NC_CACHE: dict = {}


def _build(reps: int = 1, **kw) -> bass.Bass:
    key = (reps, tuple(sorted(kw.items())))
    if key not in _NC_CACHE:
        _NC_CACHE[key] = _build_nc(reps, **kw)
    return _NC_CACHE[key]


def _in_maps(x, w, b):
    x16 = np.asarray(x, dtype=np.float16)
    w16 = np.ascontiguousarray(np.asarray(w, dtype=np.float16))
    b = np.ascontiguousarray(np.asarray(b, dtype=np.float32))
    return [
        {
            "x": x16[c * BPC : (c + 1) * BPC],
            "w": w16,
            "bias": b,
            "bias_s": b / D,
        }
        for c in range(N_CORES)
    ]


def kernel(**inputs):
    x = np.asarray(inputs["inputs"], dtype=np.float32)
    w = np.asarray(inputs["att_weight"], dtype=np.float32)
    b = np.asarray(inputs["att_bias"], dtype=np.float32)
    nc = _build()
    res = run_bass_kernel_spmd(nc, _in_maps(x, w, b), list(range(N_CORES)))
    return np.concatenate([r["out"] for r in res.results], axis=0)


# revision 11
# speedup vs baseline: 2.7637x; 1.7699x over previous
"""Attention-pooling kernel for Trainium2 (raw Bass), SPMD over 8 NeuronCores.

Computation (per sample b):
    score[t] = tanh(sum_d X[b,t,d] * w[d] + bias[t])
    out[b,d] = sum_t softmax_t(score)[t] * X[b,t,d]

Sharding: data-parallel over batch (32 samples -> 4 per core); w/bias
replicated. X uploaded fp16 (t-rows permuted t = p*16 + c so per-sample DMA
slices are contiguous 16KB-per-partition runs; softmax/pooling are
t-permutation invariant, bias is loaded with the same permutation).

v3 design (HW-measured on this container's trn2 cores):
  - The 16-chunk score multiply+reduce per sample is split across THREE
    engines (HW-probed costs: DVE stt ~1.0-1.13us/chunk fused; DVE TT 2x
    ~0.5us + ACT Identity-accum ~1.1-1.23us; Pool TT mult ~1.74us/chunk):
      g0: stt{0-4} on DVE, Pool-TT{5,6}, DVE-TT{7}
      g1: stt{8-11} on DVE, Pool-TT{12,13,14}, DVE-TT{15}
    ACT reduces all Pool/DVE TT products (Identity with bias/D folded in as
    the per-element ACT bias). Balances DVE ~10.3 / ACT ~9.8 / Pool ~8.7 us
    per sample against the HW DMA floor of ~10.9us/sample (~390 GB/s/core,
    measured: the 16MiB fp16 X load is the hard roofline; neither SWDGE vs
    HWDGE issue nor deeper buffering changes it).
  - Input DMA issue moved from Pool (busy computing now) to the SP engine,
    which does NOTHING else (an output DMA on the SP queue would gate input
    issue on the downstream pipeline — measured as full serialization in the
    timeline sim). Output DMAs are issued from the ACT queue right after the
    copies that produce them, gated on the copies' completion semaphore.
  - 4-deep xt buffering (the issue->transfer->score->pool loop spans ~36us;
    3 buffers would cap the pipeline at 47us/pass, below the DMA rate).
  - score/esc/sumexp/recip triple-buffered so softmax(v) need not wait for
    pooling(v-2), only pooling(v-3).
  - Pooling interleaved per group on PE as in v2 (16 matmuls after each
    group's exp); 1/sum(exp) folded into the ACT PSUM->SBUF output copies
    as the per-partition scale.
  - An explicit self-semaphore wait remains between DVE stt and the next
    DVE reader of its accum_out (accum lands after retire; race confirmed
    on HW by the v2 session).
Steady state targets the DMA roofline ~44us per pass of 4 samples (v2: 57us).
"""

import numpy as np

import concourse.bass as bass
import concourse.mybir as mybir
from concourse.bass_utils import run_bass_kernel_spmd

B, T, D = 32, 2048, 1024
N_CORES = 8
BPC = B // N_CORES  # samples per core
P = 128
NCHUNK = T // P  # 16
NGROUP = 2
CPG = NCHUNK // NGROUP  # 8

f32 = mybir.dt.float32
fp16 = mybir.dt.float16
Tanh = mybir.ActivationFunctionType.Tanh
Exp = mybir.ActivationFunctionType.Exp
Copy = mybir.ActivationFunctionType.Copy
Identity = mybir.ActivationFunctionType.Identity
Alu = mybir.AluOpType

# per-group chunk assignment: S on DVE (fused stt), Q on Pool (TT mult),
# C on DVE (TT mult); Q and C products reduced on ACT.
S0, Q0, C0 = [0, 1, 2, 3, 4], [5, 6], [7]
S1, Q1, C1 = [8, 9, 10, 11], [12, 13, 14], [15]
NQ = len(Q0) + len(Q1)  # Pool chunks / sample
NC = len(C0) + len(C1)  # DVE-TT chunks / sample
PSLOT = 6  # Pool product ring depth
DSLOT = 3  # DVE product ring depth
DSLOT_NP = 10  # DVE ring depth when pool_q disabled
XBUF = 4  # xt buffers
SBUF3 = 3  # score/esc/sumexp/recip buffers


def _build_nc(reps: int = 1, pool_q: bool = False, act_dma: bool = True, in_dma: bool = True) -> bass.Bass:
    nc = bass.Bass("TRN2", target_bir_lowering=False, debug=False)
    x = nc.dram_tensor("x", [BPC, T, D], fp16, kind="ExternalInput").ap()
    w = nc.dram_tensor("w", [D, 1], fp16, kind="ExternalInput")
    bias = nc.dram_tensor("bias", [T, 1], f32, kind="ExternalInput")
    bias_s = nc.dram_tensor("bias_s", [T, 1], f32, kind="ExternalInput")  # bias/D
    bias_w = nc.dram_tensor("bias_w", [T, 1], f32, kind="ExternalInput")  # bias/sum(w)
    out = nc.dram_tensor("out", [BPC, D], f32, kind="ExternalOutput").ap()

    NS = BPC * reps
    if pool_q:
        s0, q0, c0, s1, q1, c1 = S0, Q0, C0, S1, Q1, C1
        dslot = DSLOT
    else:
        s0, q0, c0 = [0, 1, 2, 3], [], [4, 5, 6, 7]
        s1, q1, c1 = [8, 9, 10], [], [11, 12, 13, 14, 15]
        dslot = DSLOT_NP
    NSTT = len(s0) + len(s1)

    from contextlib import ExitStack

    with ExitStack() as es:
        ec = es.enter_context
        xt = [ec(nc.sbuf_tensor(f"xt{i}", [P, NCHUNK, D], fp16)) for i in range(XBUF)]
        wt = ec(nc.sbuf_tensor("wt", [P, D], fp16))
        bias_t = ec(nc.sbuf_tensor("bias_t", [P, NCHUNK], f32))
        bias_st = ec(nc.sbuf_tensor("bias_st", [P, NCHUNK], f32))
        bias_wt = ec(nc.sbuf_tensor("bias_wt", [P, NCHUNK], f32))
        ones_col = ec(nc.sbuf_tensor("ones_col", [P, 1], f32))
        prodp = ec(nc.sbuf_tensor("prodp", [P, PSLOT, D], fp16))
        prodd = ec(nc.sbuf_tensor("prodd", [P, dslot, D], fp16))
        scrap = ec(nc.sbuf_tensor("scrap", [P, NCHUNK], fp16))
        score = [
            ec(nc.sbuf_tensor(f"score{i}", [P, NCHUNK], f32)) for i in range(SBUF3)
        ]
        esc = [ec(nc.sbuf_tensor(f"esc{i}", [P, NCHUNK], fp16)) for i in range(SBUF3)]
        sumexp = [
            ec(nc.sbuf_tensor(f"sumexp{i}", [P, NGROUP], f32)) for i in range(SBUF3)
        ]
        recip = [ec(nc.sbuf_tensor(f"recip{i}", [1, 1], f32)) for i in range(SBUF3)]
        orow = [ec(nc.sbuf_tensor(f"orow{i}", [1, BPC * D], f32)) for i in range(2)]
        pa = [ec(nc.psum_tensor(f"pool_a{i}", [1, 512], f32)) for i in range(2)]
        pb = [ec(nc.psum_tensor(f"pool_b{i}", [1, 512], f32)) for i in range(2)]
        tot = [ec(nc.psum_tensor(f"tot{i}", [1, 1], f32)) for i in range(2)]

        cset = ec(nc.semaphore("cset"))
        ones_sem = ec(nc.semaphore("ones_sem"))
        dma_in_s = [
            [ec(nc.semaphore(f"dma_in{b_}{g}")) for g in range(NGROUP)]
            for b_ in range(XBUF)
        ]
        poolp = ec(nc.semaphore("poolp"))  # Pool TT products done
        dved = ec(nc.semaphore("dved"))  # DVE TT products done
        sttb = ec(nc.semaphore("sttb"))  # DVE stt accum landed
        qb = ec(nc.semaphore("qb"))  # bias added per group
        actq = ec(nc.semaphore("actq"))  # ACT reduces of Pool products
        actc = ec(nc.semaphore("actc"))  # ACT reduces of DVE products
        act_g = ec(nc.semaphore("act_g"))  # exp done (per group: 2/sample)
        pe_tot = ec(nc.semaphore("pe_tot"))  # tot matmul done (1/sample)
        recip_sem = ec(nc.semaphore("recip_sem"))  # recip ready (1/sample)
        act_out = ec(nc.semaphore("act_out"))  # output copies done (1/sample)
        pe_pool = ec(nc.semaphore("pe_pool"))  # pooling done (1/sample)
        dma_out_s = [ec(nc.semaphore(f"dma_out{i}")) for i in range(2)]
        block = ec(nc.Block())

        def k_of(v):  # how many times xt slot v%XBUF has been (re)loaded
            return (v // XBUF + 1) if in_dma else 1

        @block.sync
        def _(sync):
            sync.dma_start(
                wt[:], bass.AP(tensor=w, offset=0, ap=[[0, P], [1, D]])
            ).then_inc(cset, 16)
            bap = [[NCHUNK, P], [1, NCHUNK]]  # bias_t[p, c] = bias[p*16 + c]
            sync.dma_start(
                bias_t[:], bass.AP(tensor=bias, offset=0, ap=bap)
            ).then_inc(cset, 16)
            sync.dma_start(
                bias_st[:], bass.AP(tensor=bias_s, offset=0, ap=bap)
            ).then_inc(cset, 16)
            sync.dma_start(
                bias_wt[:], bass.AP(tensor=bias_w, offset=0, ap=bap)
            ).then_inc(cset, 16)
            for v in range(NS if in_dma else min(NS, XBUF)):
                s4 = v % XBUF
                if v >= XBUF:
                    sync.wait_ge(pe_pool, v - XBUF + 1)  # xt slot free
                xs = x[v % BPC].rearrange("(p c) d -> p c d", c=NCHUNK)
                for g in range(NGROUP):
                    sync.dma_start(
                        out=xt[s4][:, g * CPG : (g + 1) * CPG, :],
                        in_=xs[:, g * CPG : (g + 1) * CPG, :],
                    ).then_inc(dma_in_s[s4][g], 16)
                if v % BPC == 0 and v >= 2 * BPC:
                    r = v // BPC - 2  # rep-wide output, 2-rep lookback
                    sync.wait_ge(act_out, BPC * (r + 1))
                    sync.dma_start(out=out[:, :], in_=orow[r % 2][:]).then_inc(
                        dma_out_s[r % 2], 16
                    )
            for r in range(max(reps - 2, 0) if in_dma else 0, reps):
                sync.wait_ge(act_out, BPC * (r + 1))
                sync.dma_start(out=out[:, :], in_=orow[r % 2][:]).then_inc(
                    dma_out_s[r % 2], 16
                )
            sync.wait_ge(dma_out_s[0], 16 * ((reps + 1) // 2))
            if reps > 1:
                sync.wait_ge(dma_out_s[1], 16 * (reps // 2))

        @block.gpsimd
        def _(gpsimd):
            gpsimd.memset(ones_col[:], 1.0).then_inc(ones_sem, 1)
            gpsimd.wait_ge(cset, 64)
            kq = 0
            for v in range(NS):
                s4 = v % XBUF
                for g, qs in ((0, q0), (1, q1)):
                    gpsimd.wait_ge(dma_in_s[s4][g], 16 * k_of(v))
                    for c in qs:
                        if kq >= PSLOT:
                            gpsimd.wait_ge(actq, kq - PSLOT + 1)
                        gpsimd.tensor_tensor(
                            out=prodp[:, kq % PSLOT, :],
                            in0=xt[s4][:, c, :],
                            in1=wt[:],
                            op=Alu.mult,
                        ).then_inc(poolp, 1)
                        kq += 1

        @block.vector
        def _(vector):
            vector.wait_ge(cset, 64)
            k_stt = 0
            kc = 0

            def recip_of(u):
                u3 = u % SBUF3
                vector.wait_ge(pe_tot, u + 1)
                if u >= SBUF3:
                    vector.wait_ge(act_out, u - SBUF3 + 1)  # recip[u3] free
                nc.vector.reciprocal(out=recip[u3][:], in_=tot[u % 2][:]).then_inc(
                    recip_sem, 1
                )

            for v in range(NS):
                s4, p3 = v % XBUF, v % SBUF3
                if v >= SBUF3:
                    # score[p3] free: exp g1 of sample v-3 done
                    vector.wait_ge(act_g, NGROUP * (v - SBUF3) + 2)
                for g, ss, cs in ((0, s0, c0), (1, s1, c1)):
                    vector.wait_ge(dma_in_s[s4][g], 16 * k_of(v))
                    for c in cs:
                        if kc >= dslot:
                            vector.wait_ge(actc, kc - dslot + 1)
                        nc.vector.tensor_tensor(
                            out=prodd[:, kc % dslot, :],
                            in0=xt[s4][:, c, :],
                            in1=wt[:],
                            op=Alu.mult,
                        ).then_inc(dved, 1)
                        kc += 1
                    for c in ss:
                        # bias folded in: (X + bias/sum(w)) * w accumulates to
                        # X.w + bias
                        nc.vector.scalar_tensor_tensor(
                            out=scrap[:, c : c + 1].broadcast_to((P, D)),
                            in0=xt[s4][:, c, :],
                            scalar=bias_wt[:, c : c + 1],
                            in1=wt[:],
                            op0=Alu.add,
                            op1=Alu.mult,
                            accum_out=score[p3][:, c : c + 1],
                        ).then_inc(sttb, 1)
                        k_stt += 1
                    if g == 0 and v >= 1:
                        recip_of(v - 1)
            recip_of(NS - 1)

        @block.scalar
        def _(scalar):
            scalar.wait_ge(cset, 64)
            kq = 0
            kc = 0

            def copies_and_out(u):
                u2, u3, s = u % 2, u % SBUF3, u % BPC
                r, rp = u // BPC, (u // BPC) % 2
                o0 = s * D
                scalar.wait_ge(pe_pool, u + 1)
                scalar.wait_ge(recip_sem, u + 1)
                if s == 0 and r >= 2:
                    # orow[rp] free: its rep-(r-2) DMA done
                    scalar.wait_ge(dma_out_s[rp], 16 * (r // 2))
                nc.scalar.activation(
                    out=orow[rp][:, o0 : o0 + 512], in_=pa[u2][:], func=Copy,
                    scale=recip[u3][:],
                )
                nc.scalar.activation(
                    out=orow[rp][:, o0 + 512 : o0 + 1024], in_=pb[u2][:], func=Copy,
                    scale=recip[u3][:],
                ).then_inc(act_out, 1)

            for v in range(NS):
                p3 = v % SBUF3
                if v >= SBUF3:
                    scalar.wait_ge(pe_pool, v - 2)  # esc/sumexp[p3] free
                for g, qs_, cs in ((0, q0, c0), (1, q1, c1)):
                    for c in qs_:
                        scalar.wait_ge(poolp, kq + 1)
                        nc.scalar.activation(
                            out=prodp[:, kq % PSLOT, :],
                            in_=prodp[:, kq % PSLOT, :],
                            func=Identity,
                            bias=bias_st[:, c : c + 1],
                            accum_out=score[p3][:, c : c + 1],
                        ).then_inc(actq, 1)
                        kq += 1
                    for c in cs:
                        scalar.wait_ge(dved, kc + 1)
                        nc.scalar.activation(
                            out=prodd[:, kc % dslot, :],
                            in_=prodd[:, kc % dslot, :],
                            func=Identity,
                            bias=bias_st[:, c : c + 1],
                            accum_out=score[p3][:, c : c + 1],
                        ).then_inc(actc, 1)
                        kc += 1
                    gs = slice(g * CPG, (g + 1) * CPG)
                    n_stt = NSTT * v + (len(s0) if g == 0 else NSTT)
                    scalar.wait_ge(sttb, n_stt)
                    nc.scalar.activation(
                        out=score[p3][:, gs], in_=score[p3][:, gs], func=Tanh
                    )
                    nc.scalar.activation(
                        out=esc[p3][:, gs],
                        in_=score[p3][:, gs],
                        func=Exp,
                        accum_out=sumexp[p3][:, g : g + 1],
                    ).then_inc(act_g, 1)
                if v >= 1:
                    copies_and_out(v - 1)
            copies_and_out(NS - 1)

        @block.tensor
        def _(tensor):
            tensor.wait_ge(ones_sem, 1)
            for v in range(NS):
                s4, p3, p2 = v % XBUF, v % SBUF3, v % 2
                for g in range(NGROUP):
                    tensor.wait_ge(act_g, NGROUP * v + g + 1)
                    if g == 0:
                        if v >= 2:
                            tensor.wait_ge(recip_sem, v - 1)  # tot[p2] free
                            tensor.wait_ge(act_out, v - 1)  # pa/pb[p2] free
                    mm_t = nc.tensor.matmul(
                        tot[p2][:],
                        sumexp[p3][:, g : g + 1],
                        ones_col[:],
                        start=(g == 0),
                        stop=(g == NGROUP - 1),
                    )
                    if g == NGROUP - 1:
                        mm_t.then_inc(pe_tot, 1)
                    for c in range(g * CPG, (g + 1) * CPG):
                        st, sp = c == 0, c == NCHUNK - 1
                        nc.tensor.matmul(
                            pa[p2][:], esc[p3][:, c : c + 1], xt[s4][:, c, 0:512],
                            start=st, stop=sp,
                        )
                        mm = nc.tensor.matmul(
                            pb[p2][:], esc[p3][:, c : c + 1], xt[s4][:, c, 512:1024],
                            start=st, stop=sp,
                        )
                mm.then_inc(pe_pool, 1)

    return nc


_NC_CACHE: dict = {}


def _build(reps: int = 1, **kw) -> bass.Bass:
    key = (reps, tuple(sorted(kw.items())))
    if key not in _NC_CACHE:
        _NC_CACHE[key] = _build_nc(reps, **kw)
    return _NC_CACHE[key]


def _in_maps(x, w, b):
    x16 = np.asarray(x, dtype=np.float16)
    w16 = np.ascontiguousarray(np.asarray(w, dtype=np.float16))
    b = np.ascontiguousarray(np.asarray(b, dtype=np.float32))
    wsum = float(np.sum(w16.astype(np.float32)))
    if abs(wsum) < 1e-6:
        wsum = 1e-6  # bias is zeros for this problem; guard division anyway
    return [
        {
            "x": x16[c * BPC : (c + 1) * BPC],
            "w": w16,
            "bias": b,
            "bias_s": b / D,
            "bias_w": b / wsum,
        }
        for c in range(N_CORES)
    ]


def kernel(**inputs):
    x = np.asarray(inputs["inputs"], dtype=np.float32)
    w = np.asarray(inputs["att_weight"], dtype=np.float32)
    b = np.asarray(inputs["att_bias"], dtype=np.float32)
    nc = _build()
    res = run_bass_kernel_spmd(nc, _in_maps(x, w, b), list(range(N_CORES)))
    return np.concatenate([r["out"] for r in res.results], axis=0)
